# revision 1
# baseline (speedup 1.0000x reference)
"""Cumulative LayerNorm Trainium2 Bass kernel.

x: [B=8, C=256, T=16000] f32.  Per timestep t: normalize x[:, :, t] by the
mean/std of all elements x[:, :, t'<=t] (cumulative over channels+time), then
scale by weight[c] and add bias[c].

Sharding: pure data parallel over B across 8 NeuronCores (1 sample/core).

Per-core algorithm (C=256 = 2 halves of 128 partitions, T on the free dim):
  Phase A (per 2000-col io-tile):
    - DMA x into SBUF (labeled f32r so the PE consumes it directly; fp32r
      truncates operands to ~13 mantissa bits inside the PE only).
    - xx = x^2 in bf16 (ACT for half 0, GPSIMD for half 1).
    - PE: s[t] = sum_c x (fp32r, exact ones weights) and sq[t] = sum_c x^2
      (bf16) as [2, 2, 512] PSUM row-blocks; evacuate row 0 to SBUF rows
      (ACT copies); DMA-reshape rows into a [128, 125] "stat layout" where
      t = 125*p + i.
  Stats (per 4000-col chunk = 32 stat rows; engine ops need 32-aligned
  partition bases):
    - DVE tensor_tensor_scan along i (per-partition prefix sums of the
      chunk's 32 rows).
    - Row totals accumulate into st[128, 2]; a strict-upper-triangular fp32r
      matmul gives exclusive cross-partition offsets (st rows of future
      chunks are zeroed, so one full-K matmul per chunk is exact).
    - mean = (scan + off) * 1/cnt (off read straight from PSUM);
      var = E[x^2] - mean^2;  istd = 1/sqrt(var + eps) (ACT Sqrt + DVE
      reciprocal);  nm = -mean.
  Phase C (per io-tile, per 1000-col half-tile):
    - Gather nm/istd stat-layout slices back into [1, 1000] rows (DMA).
    - PE rank-1 broadcasts: nm_bc = ones x nm_row (PSUM),
      ibc = ones x istd_row (PSUM, copied to SBUF on ACT).
    - DVE scalar_tensor_tensor pair per half: z = nm_bc + x;
      y = (z * w[p]) * istd_bc; DMA out.

Emission is software-pipelined at io-tile granularity (phase C lags phase A
by 3 tiles) so the strict-FIFO engine queues always hold ready work ahead of
the long-latency stats chain.
"""
import ml_dtypes
import numpy as np

B, C, T = 8, 256, 16000
P = 128
NH = 2                     # channel halves
CHUNK = 2000               # t per io-tile
NCHUNK = T // CHUNK        # 8
ROWS = T // P              # 125  (stat layout free dim; t = 125*p + i)
PB = 500                   # psum block columns (4 per io-tile)
NPB = CHUNK // PB          # 4
EPS = 1e-06

_cached = {}


def _build_nc(with_bias: bool):
    from contextlib import ExitStack

    import concourse.tile as tile
    from concourse import bacc, mybir

    f32 = mybir.dt.float32
    f32r = mybir.dt.float32r
    bf16 = mybir.dt.bfloat16
    ALU = mybir.AluOpType
    ACTF = mybir.ActivationFunctionType

    nc = bacc.Bacc()

    x = nc.dram_tensor("x", [C, T], f32, kind="ExternalInput")
    wvec = nc.dram_tensor("wvec", [C, 1], f32, kind="ExternalInput")
    iden_d = nc.dram_tensor("iden", [P, P], f32r, kind="ExternalInput")
    tri_d = nc.dram_tensor("tri", [P, P], f32r, kind="ExternalInput")
    ones2r_d = nc.dram_tensor("ones2r", [P, 2], f32r, kind="ExternalInput")
    ones2b_d = nc.dram_tensor("ones2b", [P, 2], bf16, kind="ExternalInput")
    onesb_d = nc.dram_tensor("onesb", [1, P], f32r, kind="ExternalInput")
    zeros2_d = nc.dram_tensor("zeros2", [P, 2], f32r, kind="ExternalInput")
    invcnt_d = nc.dram_tensor("invcnt", [P, ROWS], f32, kind="ExternalInput")
    if with_bias:
        bvec = nc.dram_tensor("bvec", [C, 1], f32, kind="ExternalInput")
    y = nc.dram_tensor("y", [C, T], f32, kind="ExternalOutput")

    with tile.TileContext(nc) as tc, ExitStack() as ctx:
        const = ctx.enter_context(tc.tile_pool(name="const", bufs=1))
        persist = ctx.enter_context(tc.tile_pool(name="persist", bufs=1))
        xpool = ctx.enter_context(tc.tile_pool(name="xpool", bufs=6))
        ypool = ctx.enter_context(tc.tile_pool(name="ypool", bufs=2))
        sqpool = ctx.enter_context(tc.tile_pool(name="sqpool", bufs=2))
        erow = ctx.enter_context(tc.tile_pool(name="erow", bufs=4))
        brow = ctx.enter_context(tc.tile_pool(name="brow", bufs=4))
        ibcsb = ctx.enter_context(tc.tile_pool(name="ibcsb", bufs=4))
        ps_s = ctx.enter_context(tc.tile_pool(name="ps_s", bufs=2, space="PSUM"))
        ps_nm = ctx.enter_context(tc.tile_pool(name="ps_nm", bufs=1, space="PSUM"))
        ps_i = ctx.enter_context(tc.tile_pool(name="ps_i", bufs=1, space="PSUM"))
        zpool = ctx.enter_context(tc.tile_pool(name="zpool", bufs=3))

        # ---- constants ----
        tri = const.tile([P, P], f32r)
        nc.sync.dma_start(out=tri, in_=tri_d[:, :])
        ones2r = const.tile([P, 2], f32r)
        nc.sync.dma_start(out=ones2r, in_=ones2r_d[:, :])
        ones2b = const.tile([P, 2], bf16)
        nc.sync.dma_start(out=ones2b, in_=ones2b_d[:, :])
        onesb = const.tile([1, P], f32r)
        nc.sync.dma_start(out=onesb, in_=onesb_d[:, :])
        invcnt = const.tile([P, ROWS], f32)
        nc.sync.dma_start(out=invcnt, in_=invcnt_d[:, :])
        w_sb = const.tile([P, NH], f32)
        for h in range(NH):
            nc.sync.dma_start(out=w_sb[:, h : h + 1], in_=wvec[h * P : (h + 1) * P, 0:1])
        if with_bias:
            b_sb = const.tile([P, NH], f32)
            for h in range(NH):
                nc.sync.dma_start(
                    out=b_sb[:, h : h + 1], in_=bvec[h * P : (h + 1) * P, 0:1]
                )
        eps_sb = const.tile([P, 1], f32)
        nc.vector.memset(eps_sb, EPS)

        # ---- persistent stat-layout surfaces ----
        s_re = persist.tile([P, ROWS], f32)     # channel sums -> prefix sums
        sq_re = persist.tile([P, ROWS], f32)
        mean_t = persist.tile([P, ROWS], f32)
        ex2_t = persist.tile([P, ROWS], f32)    # E[x^2] -> var
        msq_t = persist.tile([P, ROWS], f32)    # mean^2 -> ln(var+eps)
        istd_t = persist.tile([P, ROWS], f32)
        nm_t = persist.tile([P, ROWS], f32)     # -mean
        st_sb = persist.tile([P, 2], f32r)      # chunk totals (s, sq)
        nc.sync.dma_start(out=st_sb, in_=zeros2_d[:, :])

        def phase_a(tix, x_t):
            """Load io-tile `tix` (2000 cols), compute channel sums/sumsq into
            stat-layout rows 16*tix .. 16*tix+16."""
            t0 = tix * CHUNK
            for h in range(NH):
                nc.sync.dma_start(
                    out=x_t[:, h, :],
                    in_=x[h * P : (h + 1) * P, t0 : t0 + CHUNK].bitcast(f32r),
                )
            xx0 = sqpool.tile([P, CHUNK], bf16, tag="xx0", name="xx0")
            nc.scalar.activation(xx0, x_t[:, 0, :].bitcast(f32), ACTF.Square)
            xx1 = sqpool.tile([P, CHUNK], bf16, tag="xx1", name="xx1")
            nc.gpsimd.tensor_tensor(
                xx1, x_t[:, 1, :].bitcast(f32), x_t[:, 1, :].bitcast(f32), ALU.mult
            )

            for a2 in range(2):  # 1000-col groups
                sps = ps_s.tile([2, 2, 512], f32, tag="stat", name="sps")
                qps = ps_s.tile([2, 2, 512], f32, tag="stat", name="qps")
                for j in range(2):
                    cs = slice((2 * a2 + j) * PB, (2 * a2 + j + 1) * PB)
                    nc.tensor.matmul(
                        sps[0:2, j, 0:PB], ones2r, x_t[:, 0, cs],
                        start=True, stop=False,
                    )
                    nc.tensor.matmul(
                        sps[0:2, j, 0:PB], ones2r, x_t[:, 1, cs],
                        start=False, stop=True,
                    )
                for j in range(2):
                    cs = slice((2 * a2 + j) * PB, (2 * a2 + j + 1) * PB)
                    nc.tensor.matmul(
                        qps[0:2, j, 0:PB], ones2b, xx0[:, cs], start=True, stop=False
                    )
                    nc.tensor.matmul(
                        qps[0:2, j, 0:PB], ones2b, xx1[:, cs], start=False, stop=True
                    )
                srow = erow.tile([1, 1024], f32, tag="erow", name="srow")
                nc.scalar.copy(
                    srow[0:1, 0:1000].rearrange("p (j n) -> p j n", j=2),
                    sps[0:1, :, 0:PB],
                )
                qrow = erow.tile([1, 1024], f32, tag="erow", name="qrow")
                nc.scalar.copy(
                    qrow[0:1, 0:1000].rearrange("p (j n) -> p j n", j=2),
                    qps[0:1, :, 0:PB],
                )
                # rows 16*tix+8*a2 .. +8 of the stat layout (t = 125*p + i)
                rp = 16 * tix + 8 * a2
                nc.sync.dma_start(out=s_re[rp : rp + 8, :], in_=srow[0:1, 0:1000])
                nc.sync.dma_start(out=sq_re[rp : rp + 8, :], in_=qrow[0:1, 0:1000])

        def stats(sc):
            """Prefix sums + mean/istd for stat-layout rows 32*sc .. 32*sc+32."""
            sl = slice(32 * sc, 32 * sc + 32)
            nc.vector.tensor_tensor_scan(
                out=s_re[sl, :], data0=s_re[sl, :], data1=s_re[sl, :],
                initial=0.0, op0=ALU.add, op1=ALU.bypass,
            )
            nc.vector.tensor_tensor_scan(
                out=sq_re[sl, :], data0=sq_re[sl, :], data1=sq_re[sl, :],
                initial=0.0, op0=ALU.add, op1=ALU.bypass,
            )
            nc.vector.tensor_copy(st_sb[sl, 0:1], s_re[sl, ROWS - 1 : ROWS])
            nc.vector.tensor_copy(st_sb[sl, 1:2], sq_re[sl, ROWS - 1 : ROWS])
            offps = ps_s.tile([P, 2], f32, tag="stat", name="offps")
            nc.tensor.matmul(offps, tri, st_sb, start=True, stop=True)

            nc.vector.scalar_tensor_tensor(
                out=mean_t[sl, :], in0=s_re[sl, :], scalar=offps[sl, 0:1],
                in1=invcnt[sl, :], op0=ALU.add, op1=ALU.mult,
            )
            nc.vector.scalar_tensor_tensor(
                out=ex2_t[sl, :], in0=sq_re[sl, :], scalar=offps[sl, 1:2],
                in1=invcnt[sl, :], op0=ALU.add, op1=ALU.mult,
            )
            nc.vector.tensor_scalar_mul(nm_t[sl, :], mean_t[sl, :], -1.0)
            nc.vector.tensor_tensor(msq_t[sl, :], mean_t[sl, :], mean_t[sl, :], ALU.mult)
            nc.vector.tensor_tensor(ex2_t[sl, :], ex2_t[sl, :], msq_t[sl, :], ALU.subtract)
            # istd = 1 / sqrt(var + eps)  (Sqrt keeps the ACT table set stable;
            # reciprocal_approx_fast is ~18 bits, far above the fp32r noise)
            nc.scalar.activation(
                msq_t[sl, :], ex2_t[sl, :], ACTF.Sqrt, bias=eps_sb[sl, :], scale=1.0
            )
            nc.vector.reciprocal(out=istd_t[sl, :], in_=msq_t[sl, :])

        def phase_c(tix, x_t):
            """Normalize io-tile `tix` and store it."""
            t0 = tix * CHUNK
            for half in range(2):  # half-tiles of 1000 columns
                rsl = slice(16 * tix + 8 * half, 16 * tix + 8 * half + 8)
                nm_row = brow.tile([1, 1024], f32r, tag="brow", name="nm_row")
                nc.sync.dma_start(
                    out=nm_row[0:1, 0:1000], in_=nm_t[rsl, :].bitcast(f32r)
                )
                istd_row = brow.tile([1, 1024], f32r, tag="brow", name="istd_row")
                nc.sync.dma_start(
                    out=istd_row[0:1, 0:1000], in_=istd_t[rsl, :].bitcast(f32r)
                )
                nm_ps = ps_nm.tile([P, 2, 512], f32, tag="nm", name="nm_ps")
                ibc = ps_i.tile([P, 2, 512], f32, tag="ibc_ps", name="ibc")
                for j in range(2):
                    cs = slice(j * PB, (j + 1) * PB)
                    nc.tensor.matmul(
                        nm_ps[:, j, 0:PB], onesb, nm_row[0:1, cs],
                        start=True, stop=True,
                    )
                    nc.tensor.matmul(
                        ibc[:, j, 0:PB], onesb, istd_row[0:1, cs], start=True, stop=True
                    )
                ibc_sb = ibcsb.tile([P, 2, 512], f32, tag="ibc", name="ibc_sb")
                nc.scalar.copy(ibc_sb[:, :, 0:PB], ibc[:, :, 0:PB])
                for h in range(NH):
                    x_ap = x_t[:, h, half * 1000 : (half + 1) * 1000].bitcast(
                        f32
                    ).rearrange("p (j n) -> p j n", j=2)
                    # z = x - mean  (one DVE op; nm_ps is the -mean broadcast)
                    z_sb = zpool.tile([P, 2, 512], f32, tag="z", name="z_sb")
                    nc.vector.scalar_tensor_tensor(
                        out=z_sb[:, :, 0:PB], in0=nm_ps[:, :, 0:PB], scalar=1.0,
                        in1=x_ap, op0=ALU.mult, op1=ALU.add,
                    )
                    # y = (z * w) * istd
                    y_t = ypool.tile([P, 2, 512], f32, tag="y", name="y_t")
                    nc.vector.scalar_tensor_tensor(
                        out=y_t[:, :, 0:PB], in0=z_sb[:, :, 0:PB],
                        scalar=w_sb[:, h : h + 1], in1=ibc_sb[:, :, 0:PB],
                        op0=ALU.mult, op1=ALU.mult,
                    )
                    if with_bias:
                        nc.vector.tensor_scalar_add(
                            out=y_t[:, :, 0:PB], in0=y_t[:, :, 0:PB],
                            scalar1=b_sb[:, h : h + 1],
                        )
                    nc.sync.dma_start(
                        out=y[h * P : (h + 1) * P,
                              t0 + half * 1000 : t0 + (half + 1) * 1000],
                        in_=y_t[:, :, 0:PB],
                    )

        # Software-pipelined emission, interleaved at io-tile granularity:
        # phase C lags phase A by ~3 tiles and A/C alternate in the emission
        # stream so every engine queue always holds ready work ahead of the
        # long-latency stats chain (strict-FIFO queues otherwise head-of-line
        # block at every chunk boundary).
        tiles = {}

        def load_a(tix):
            x_t = xpool.tile([P, NH, CHUNK], f32r, tag="x", name="x_t")
            phase_a(tix, x_t)
            tiles[tix] = x_t

        na = nc_done = 0
        for tix in range(3):
            load_a(tix)
        na = 3
        while nc_done < NCHUNK:
            if nc_done % 2 == 0:
                stats(nc_done // 2)
            phase_c(nc_done, tiles.pop(nc_done))
            nc_done += 1
            if na < NCHUNK:
                load_a(na)
                na += 1
    nc.compile()
    return nc


def _consts():
    iden = np.eye(P, dtype=np.float32)
    tri = np.triu(np.ones((P, P), dtype=np.float32), k=1)  # tri[k,m]=1 iff k<m
    ones2 = np.ones((P, 2), dtype=np.float32)
    onesb = np.ones((1, P), dtype=np.float32)
    t_idx = (125 * np.arange(P, dtype=np.float64)[:, None]
             + np.arange(ROWS, dtype=np.float64)[None, :])
    invcnt = (1.0 / (C * (t_idx + 1.0))).astype(np.float32)
    return {"iden": iden, "tri": tri, "ones2r": ones2,
            "ones2b": ones2.astype(ml_dtypes.bfloat16), "onesb": onesb,
            "zeros2": np.zeros((P, 2), dtype=np.float32), "invcnt": invcnt}


def _get_nc(with_bias: bool):
    key = ("nc", with_bias)
    if key not in _cached:
        _cached[key] = _build_nc(with_bias)
    return _cached[key]


def _run(x, weight, bias, trace=False):
    from concourse.bass_utils import run_bass_kernel_spmd

    x = np.ascontiguousarray(np.asarray(x, dtype=np.float32))
    weight = np.asarray(weight, dtype=np.float32).reshape(C, 1)
    bias = np.asarray(bias, dtype=np.float32).reshape(C, 1)
    with_bias = bool(np.any(bias))
    nc = _get_nc(with_bias)

    consts = _consts()
    in_maps = []
    for b in range(B):
        m = {"x": np.ascontiguousarray(x[b]), "wvec": weight}
        if with_bias:
            m["bvec"] = bias
        m.update(consts)
        in_maps.append(m)

    res = run_bass_kernel_spmd(nc, in_maps, core_ids=list(range(B)), trace=trace)
    y = np.stack([r["y"] for r in res.results], axis=0)
    return y, res


def kernel(x, weight, bias):
    y, _ = _run(x, weight, bias, trace=False)
    return y



# revision 4
# speedup vs baseline: 1.1923x; 1.1923x over previous
"""Cumulative LayerNorm Trainium2 Bass kernel.

x: [B=8, C=256, T=16000] f32.  Per timestep t: normalize x[:, :, t] by the
mean/std of all elements x[:, :, t'<=t] (cumulative over channels+time), then
scale by weight[c] and add bias[c].

Sharding: pure data parallel over B across 8 NeuronCores (1 sample/core).

Per-core algorithm (C=256 = 2 halves of 128 partitions, T on the free dim):
  Phase A (per 2000-col io-tile):
    - One 3D DMA loads both channel halves into SBUF [128, 2, 2000]
      (labeled f32r so the PE consumes it directly; fp32r truncates operands
      to ~13 mantissa bits inside the PE only).
    - xx = x^2 in bf16 (ACT for half 0, GPSIMD for half 1).
    - PE: s[t] = sum_c x (fp32r ones weights) and sq[t] = sum_c x^2 (bf16)
      as [2, 2, 512] PSUM row-blocks per 1000-col group; ACT evacuates row 0
      to a [1, 1000] SBUF row; a reshape DMA (issued from GPSIMD, cheap
      there) scatters it into the [128, 125] "stat layout" where t = 125p+i.
  Stats (per 4000-col chunk = 32 stat rows; engine ops need 32-aligned
  partition bases):
    - DVE tensor_tensor_scan along i (per-partition prefix sums).
    - Row totals go to st[128, 2]; a strict-upper-triangular fp32r matmul
      gives exclusive cross-partition offsets (future rows zeroed, so one
      full-K matmul per chunk is exact).
    - nm = -(scan + off) * invcnt directly (negation folded into the
      constant); var = E[x^2] - nm^2; istd = 1/sqrt(var + eps) (ACT Sqrt +
      DVE reciprocal).  nm and istd land in one [128, 2, 125] surface.
  Phase C (per io-tile):
    - ONE gather DMA (issued from ACT right after the chunk chain) pulls the
      tile's 16 stat rows into a row buffer brow[1, 16, 2, 125].
    - Per 1000-col half-tile: PE rank-1 broadcasts nm_bc and ibc into PSUM.
    - DVE scalar_tensor_tensor pair per channel half: z = nm_bc + x;
      y = (z * w[p]) * ibc -- the second STT reads ibc straight from PSUM
      (no ACT copy).  y lands in a per-(tile, half) [128, 2000] staging tile
      stored with one DMA.

Emission is software-pipelined at io-tile granularity (phase C lags the
x-loads by 4 tiles) and each engine's FIFO is ordered so streaming work never
queues behind the long-latency stats chain.
"""
import ml_dtypes
import numpy as np

B, C, T = 8, 256, 16000
P = 128
NH = 2                     # channel halves
CHUNK = 2000               # t per io-tile
NCHUNK = T // CHUNK        # 8
ROWS = T // P              # 125  (stat layout free dim; t = 125*p + i)
PB = 500                   # psum block columns (4 per io-tile)
LAG = 4                    # x-load leads phase C by this many tiles
EPS = 1e-06

_cached = {}


def _build_nc(with_bias: bool):
    from contextlib import ExitStack

    import concourse.tile as tile
    from concourse import bacc, mybir

    f32 = mybir.dt.float32
    f32r = mybir.dt.float32r
    bf16 = mybir.dt.bfloat16
    ALU = mybir.AluOpType
    ACTF = mybir.ActivationFunctionType

    nc = bacc.Bacc()

    x = nc.dram_tensor("x", [C, T], f32, kind="ExternalInput")
    wvec = nc.dram_tensor("wvec", [C, 1], f32, kind="ExternalInput")
    tri_d = nc.dram_tensor("tri", [P, P], f32r, kind="ExternalInput")
    ones2r_d = nc.dram_tensor("ones2r", [P, 2], f32r, kind="ExternalInput")
    ones2b_d = nc.dram_tensor("ones2b", [P, 2], bf16, kind="ExternalInput")
    onesb_d = nc.dram_tensor("onesb", [1, P], f32r, kind="ExternalInput")
    zeros2_d = nc.dram_tensor("zeros2", [P, 2], f32r, kind="ExternalInput")
    invcnt_d = nc.dram_tensor("invcnt", [P, ROWS], f32, kind="ExternalInput")
    ninvcnt_d = nc.dram_tensor("ninvcnt", [P, ROWS], f32, kind="ExternalInput")
    if with_bias:
        bvec = nc.dram_tensor("bvec", [C, 1], f32, kind="ExternalInput")
    y = nc.dram_tensor("y", [C, T], f32, kind="ExternalOutput")

    with tile.TileContext(nc) as tc, ExitStack() as ctx:
        const = ctx.enter_context(tc.tile_pool(name="const", bufs=1))
        persist = ctx.enter_context(tc.tile_pool(name="persist", bufs=1))
        xpool = ctx.enter_context(tc.tile_pool(name="xpool", bufs=LAG + 1))
        ypool = ctx.enter_context(tc.tile_pool(name="ypool", bufs=2))
        sqpool = ctx.enter_context(tc.tile_pool(name="sqpool", bufs=2))
        erow = ctx.enter_context(tc.tile_pool(name="erow", bufs=3))
        brpool = ctx.enter_context(tc.tile_pool(name="brow", bufs=2))
        ps_s = ctx.enter_context(tc.tile_pool(name="ps_s", bufs=2, space="PSUM"))
        ps_nm = ctx.enter_context(tc.tile_pool(name="ps_nm", bufs=1, space="PSUM"))
        ps_i = ctx.enter_context(tc.tile_pool(name="ps_i", bufs=1, space="PSUM"))
        zpool = ctx.enter_context(tc.tile_pool(name="zpool", bufs=3))

        # ---- constants ----
        tri = const.tile([P, P], f32r)
        nc.sync.dma_start(out=tri, in_=tri_d[:, :])
        ones2r = const.tile([P, 2], f32r)
        nc.sync.dma_start(out=ones2r, in_=ones2r_d[:, :])
        ones2b = const.tile([P, 2], bf16)
        nc.sync.dma_start(out=ones2b, in_=ones2b_d[:, :])
        onesb = const.tile([1, P], f32r)
        nc.sync.dma_start(out=onesb, in_=onesb_d[:, :])
        invcnt = const.tile([P, ROWS], f32)
        nc.sync.dma_start(out=invcnt, in_=invcnt_d[:, :])
        ninvcnt = const.tile([P, ROWS], f32)
        nc.sync.dma_start(out=ninvcnt, in_=ninvcnt_d[:, :])
        w_sb = const.tile([P, NH], f32)
        for h in range(NH):
            nc.sync.dma_start(out=w_sb[:, h : h + 1], in_=wvec[h * P : (h + 1) * P, 0:1])
        if with_bias:
            b_sb = const.tile([P, NH], f32)
            for h in range(NH):
                nc.sync.dma_start(
                    out=b_sb[:, h : h + 1], in_=bvec[h * P : (h + 1) * P, 0:1]
                )
        eps_sb = const.tile([P, 1], f32)
        nc.vector.memset(eps_sb, EPS)

        # ---- persistent stat-layout surfaces ----
        s_re = persist.tile([P, ROWS], f32)     # channel sums -> prefix sums
        sq_re = persist.tile([P, ROWS], f32)
        nmist = persist.tile([P, 2, ROWS], f32)  # plane 0: -mean, plane 1: istd
        ex2_t = persist.tile([P, ROWS], f32)    # E[x^2] -> var
        msq_t = persist.tile([P, ROWS], f32)    # mean^2 -> sqrt(var+eps)
        st_sb = persist.tile([P, 2], f32r)      # chunk totals (s, sq)
        nc.sync.dma_start(out=st_sb, in_=zeros2_d[:, :])

        tiles = {}

        def load_x(tix):
            """One 3D DMA for both halves of io-tile tix (issued from SP)."""
            t0 = tix * CHUNK
            x_t = xpool.tile([P, NH, CHUNK], f32r, tag="x", name="x_t")
            nc.sync.dma_start(
                out=x_t,
                in_=x.rearrange("(h p) t -> p h t", h=NH)[
                    :, :, t0 : t0 + CHUNK
                ].bitcast(f32r),
            )
            tiles[tix] = x_t

        def phase_a(tix):
            """Squares + channel sums/sumsq of io-tile tix into stat rows."""
            x_t = tiles[tix]
            xx0 = sqpool.tile([P, CHUNK], bf16, tag="xx0", name="xx0")
            nc.scalar.activation(xx0, x_t[:, 0, :].bitcast(f32), ACTF.Square)
            xx1 = sqpool.tile([P, CHUNK], bf16, tag="xx1", name="xx1")
            nc.gpsimd.tensor_tensor(
                xx1, x_t[:, 1, :].bitcast(f32), x_t[:, 1, :].bitcast(f32), ALU.mult
            )

            for a2 in range(2):  # 1000-col groups
                sps = ps_s.tile([2, 2, 512], f32, tag="stat", name="sps")
                qps = ps_s.tile([2, 2, 512], f32, tag="stat", name="qps")
                for j in range(2):
                    cs = slice((2 * a2 + j) * PB, (2 * a2 + j + 1) * PB)
                    nc.tensor.matmul(
                        sps[0:2, j, 0:PB], ones2r, x_t[:, 0, cs],
                        start=True, stop=False,
                    )
                    nc.tensor.matmul(
                        sps[0:2, j, 0:PB], ones2r, x_t[:, 1, cs],
                        start=False, stop=True,
                    )
                for j in range(2):
                    cs = slice((2 * a2 + j) * PB, (2 * a2 + j + 1) * PB)
                    nc.tensor.matmul(
                        qps[0:2, j, 0:PB], ones2b, xx0[:, cs], start=True, stop=False
                    )
                    nc.tensor.matmul(
                        qps[0:2, j, 0:PB], ones2b, xx1[:, cs], start=False, stop=True
                    )
                srow = erow.tile([1, 1024], f32, tag="erow", name="srow")
                nc.scalar.copy(
                    srow[0:1, 0:1000].rearrange("p (j n) -> p j n", j=2),
                    sps[0:1, :, 0:PB],
                )
                qrow = erow.tile([1, 1024], f32, tag="erow", name="qrow")
                nc.scalar.copy(
                    qrow[0:1, 0:1000].rearrange("p (j n) -> p j n", j=2),
                    qps[0:1, :, 0:PB],
                )
                # rows 16*tix+8*a2 .. +8 of the stat layout (t = 125*p + i);
                # reshape DMAs are cheap to issue from the GPSIMD queue.
                rp = 16 * tix + 8 * a2
                nc.gpsimd.dma_start(out=s_re[rp : rp + 8, :], in_=srow[0:1, 0:1000])
                nc.gpsimd.dma_start(out=sq_re[rp : rp + 8, :], in_=qrow[0:1, 0:1000])

        def stats(sc):
            """Prefix sums + nm/istd for stat-layout rows 32*sc .. 32*sc+32."""
            sl = slice(32 * sc, 32 * sc + 32)
            nc.vector.tensor_tensor_scan(
                out=s_re[sl, :], data0=s_re[sl, :], data1=s_re[sl, :],
                initial=0.0, op0=ALU.add, op1=ALU.bypass,
            )
            nc.vector.tensor_tensor_scan(
                out=sq_re[sl, :], data0=sq_re[sl, :], data1=sq_re[sl, :],
                initial=0.0, op0=ALU.add, op1=ALU.bypass,
            )
            nc.vector.tensor_copy(st_sb[sl, 0:1], s_re[sl, ROWS - 1 : ROWS])
            nc.vector.tensor_copy(st_sb[sl, 1:2], sq_re[sl, ROWS - 1 : ROWS])
            offps = ps_s.tile([P, 2], f32, tag="stat", name="offps")
            nc.tensor.matmul(offps, tri, st_sb, start=True, stop=True)

            # nm = -(s + off) / cnt  (negation folded into the constant)
            nc.vector.scalar_tensor_tensor(
                out=nmist[sl, 0, :], in0=s_re[sl, :], scalar=offps[sl, 0:1],
                in1=ninvcnt[sl, :], op0=ALU.add, op1=ALU.mult,
            )
            nc.vector.scalar_tensor_tensor(
                out=ex2_t[sl, :], in0=sq_re[sl, :], scalar=offps[sl, 1:2],
                in1=invcnt[sl, :], op0=ALU.add, op1=ALU.mult,
            )
            nc.vector.tensor_tensor(
                msq_t[sl, :], nmist[sl, 0, :], nmist[sl, 0, :], ALU.mult
            )
            nc.vector.tensor_tensor(ex2_t[sl, :], ex2_t[sl, :], msq_t[sl, :], ALU.subtract)
            # istd = 1 / sqrt(var + eps)  (Sqrt keeps the ACT table set stable)
            nc.scalar.activation(
                msq_t[sl, :], ex2_t[sl, :], ACTF.Sqrt, bias=eps_sb[sl, :], scale=1.0
            )
            nc.vector.reciprocal(out=nmist[sl, 1, :], in_=msq_t[sl, :])

        def gather(tix):
            """Two DMAs: the tile's 16 nm/istd stat rows -> brow planes
            [1, 2, 16, 125] (plane-major so broadcast rhs slices stay
            contiguous).  Issued from ACT, right behind the chunk's Sqrt."""
            rsl = slice(16 * tix, 16 * tix + 16)
            brow = brpool.tile([1, 2, 16, ROWS], f32, tag="brow", name="brow")
            nc.scalar.dma_start(out=brow[:, 0, :, :], in_=nmist[rsl, 0, :])
            nc.scalar.dma_start(out=brow[:, 1, :, :], in_=nmist[rsl, 1, :])
            return brow

        def half_c(tix, brow, half, y_st):
            """Broadcast + apply for 1000-col half-tile `half` of io-tile."""
            x_t = tiles[tix]
            nm_ps = ps_nm.tile([P, 2, 512], f32, tag="nm", name="nm_ps")
            ibc = ps_i.tile([P, 2, 512], f32, tag="ibc_ps", name="ibc")
            for j in range(2):
                r0 = 8 * half + 4 * j
                nc.tensor.matmul(
                    nm_ps[:, j, 0:PB], onesb,
                    brow[0:1, 0, r0 : r0 + 4, :].bitcast(f32r),
                    start=True, stop=True,
                )
                nc.tensor.matmul(
                    ibc[:, j, 0:PB], onesb,
                    brow[0:1, 1, r0 : r0 + 4, :].bitcast(f32r),
                    start=True, stop=True,
                )
            for h in range(NH):
                x_ap = x_t[:, h, half * 1000 : (half + 1) * 1000].bitcast(
                    f32
                ).rearrange("p (j n) -> p j n", j=2)
                # z = x - mean  (one DVE op; nm_ps is the -mean broadcast)
                z_sb = zpool.tile([P, 2, PB], f32, tag="z", name="z_sb")
                nc.vector.scalar_tensor_tensor(
                    out=z_sb, in0=nm_ps[:, :, 0:PB], scalar=1.0,
                    in1=x_ap, op0=ALU.mult, op1=ALU.add,
                )
                # y = (z * w) * istd, istd read straight from PSUM
                y_ap = y_st[h][
                    :, half * 1000 : (half + 1) * 1000
                ].rearrange("p (j n) -> p j n", j=2)
                nc.vector.scalar_tensor_tensor(
                    out=y_ap, in0=z_sb, scalar=w_sb[:, h : h + 1],
                    in1=ibc[:, :, 0:PB], op0=ALU.mult, op1=ALU.mult,
                )
                if with_bias:
                    nc.vector.tensor_scalar_add(
                        out=y_ap, in0=y_ap, scalar1=b_sb[:, h : h + 1]
                    )

        # ---- software-pipelined emission ----
        # Per-engine FIFO ordering is chosen so streaming work (x-loads,
        # squares, sums) never queues behind the serial stats chain:
        #   SP:   x-load(k+LAG), y-stores(k)
        #   PE:   tri(sc), C(k) halfA mms, A(k+LAG) mms, C(k) halfB mms
        #   ACT:  sqrt(sc), gather(k), xx0(k+LAG), evacs(k+LAG)
        #   DVE:  chain(sc), applies halfA, applies halfB
        #   Pool: xx1(k+LAG), stat DMAs(k+LAG)
        for t in range(LAG):
            load_x(t)
            phase_a(t)
        for k in range(NCHUNK):
            if k % 2 == 0:
                stats(k // 2)
            brow = gather(k)
            y_st = {}
            for h in range(NH):
                y_st[h] = ypool.tile([P, CHUNK], f32, tag=f"y{h}", name=f"y{h}")
            half_c(k, brow, 0, y_st)
            if k + LAG < NCHUNK:
                load_x(k + LAG)
                phase_a(k + LAG)
            half_c(k, brow, 1, y_st)
            tiles.pop(k)
            t0 = k * CHUNK
            for h in range(NH):
                nc.sync.dma_start(
                    out=y[h * P : (h + 1) * P, t0 : t0 + CHUNK], in_=y_st[h]
                )
    nc.compile()
    return nc


def _consts():
    tri = np.triu(np.ones((P, P), dtype=np.float32), k=1)  # tri[k,m]=1 iff k<m
    ones2 = np.ones((P, 2), dtype=np.float32)
    onesb = np.ones((1, P), dtype=np.float32)
    t_idx = (125 * np.arange(P, dtype=np.float64)[:, None]
             + np.arange(ROWS, dtype=np.float64)[None, :])
    invcnt = (1.0 / (C * (t_idx + 1.0))).astype(np.float32)
    return {"tri": tri, "ones2r": ones2,
            "ones2b": ones2.astype(ml_dtypes.bfloat16), "onesb": onesb,
            "zeros2": np.zeros((P, 2), dtype=np.float32),
            "invcnt": invcnt, "ninvcnt": -invcnt}


def _get_nc(with_bias: bool):
    key = ("nc", with_bias)
    if key not in _cached:
        _cached[key] = _build_nc(with_bias)
    return _cached[key]


def _run(x, weight, bias, trace=False):
    from concourse.bass_utils import run_bass_kernel_spmd

    x = np.ascontiguousarray(np.asarray(x, dtype=np.float32))
    weight = np.asarray(weight, dtype=np.float32).reshape(C, 1)
    bias = np.asarray(bias, dtype=np.float32).reshape(C, 1)
    with_bias = bool(np.any(bias))
    nc = _get_nc(with_bias)

    consts = _consts()
    in_maps = []
    for b in range(B):
        m = {"x": np.ascontiguousarray(x[b]), "wvec": weight}
        if with_bias:
            m["bvec"] = bias
        m.update(consts)
        in_maps.append(m)

    res = run_bass_kernel_spmd(nc, in_maps, core_ids=list(range(B)), trace=trace)
    y = np.stack([r["y"] for r in res.results], axis=0)
    return y, res


def kernel(x, weight, bias):
    y, _ = _run(x, weight, bias, trace=False)
    return y


# revision 7
# speedup vs baseline: 1.2047x; 1.0104x over previous
"""Cumulative LayerNorm Trainium2 Bass kernel.

x: [B=8, C=256, T=16000] f32.  Per timestep t: normalize x[:, :, t] by the
mean/std of all elements x[:, :, t'<=t] (cumulative over channels+time), then
scale by weight[c] and add bias[c].

Sharding: pure data parallel over B across 8 NeuronCores (1 sample/core).

Per-core algorithm (C=256 = 2 halves of 128 partitions, T on the free dim):
  Phase A (per 2000-col io-tile):
    - One 3D DMA loads both channel halves into SBUF [128, 2, 2000]
      (labeled f32r so the PE consumes it directly; fp32r truncates operands
      to ~13 mantissa bits inside the PE only).
    - xx = x^2 in bf16 (ACT for half 0, GPSIMD for half 1).
    - PE: s[t] = sum_c x (fp32r ones weights) and sq[t] = sum_c x^2 (bf16)
      as [2, 2, 512] PSUM row-blocks per 1000-col group; ACT evacuates row 0
      to a [1, 1000] SBUF row; a reshape DMA (issued from GPSIMD, cheap
      there) scatters it into the [128, 125] "stat layout" where t = 125p+i.
  Stats (per 4000-col chunk = 32 stat rows; engine ops need 32-aligned
  partition bases):
    - DVE tensor_tensor_scan along i (per-partition prefix sums).
    - Row totals go to st[128, 2]; a strict-upper-triangular fp32r matmul
      gives exclusive cross-partition offsets (future rows zeroed, so one
      full-K matmul per chunk is exact).
    - nm = -(scan + off) * invcnt directly (negation folded into the
      constant); var = E[x^2] - nm^2; istd = 1/sqrt(var + eps) (ACT Sqrt +
      DVE reciprocal).  nm and istd land in one [128, 2, 125] surface.
  Phase C (per io-tile):
    - ONE gather DMA (issued from ACT right after the chunk chain) pulls the
      tile's 16 stat rows into a row buffer brow[1, 16, 2, 125].
    - Per 1000-col half-tile: PE rank-1 broadcasts nm_bc and ibc into PSUM.
    - DVE scalar_tensor_tensor pair per channel half: z = nm_bc + x;
      y = (z * w[p]) * ibc -- the second STT reads ibc straight from PSUM
      (no ACT copy).  y lands in a per-(tile, half) [128, 2000] staging tile
      stored with one DMA.

Emission is software-pipelined at io-tile granularity (phase C lags the
x-loads by 4 tiles) and each engine's FIFO is ordered so streaming work never
queues behind the long-latency stats chain.
"""
import ml_dtypes
import numpy as np

B, C, T = 8, 256, 16000
P = 128
NH = 2                     # channel halves
CHUNK = 2000               # t per io-tile
NCHUNK = T // CHUNK        # 8
ROWS = T // P              # 125  (stat layout free dim; t = 125*p + i)
PB = 500                   # psum block columns (4 per io-tile)
LAG = 4                    # x-load leads phase C by this many tiles
EPS = 1e-06

_cached = {}


def _build_nc(with_bias: bool):
    from contextlib import ExitStack

    import concourse.tile as tile
    from concourse import bacc, mybir

    f32 = mybir.dt.float32
    f32r = mybir.dt.float32r
    bf16 = mybir.dt.bfloat16
    ALU = mybir.AluOpType
    ACTF = mybir.ActivationFunctionType

    nc = bacc.Bacc()

    x = nc.dram_tensor("x", [C, T], f32, kind="ExternalInput")
    wvec = nc.dram_tensor("wvec", [C, 1], f32, kind="ExternalInput")
    tri_d = nc.dram_tensor("tri", [P, P], f32r, kind="ExternalInput")
    ones2r_d = nc.dram_tensor("ones2r", [P, 2], f32r, kind="ExternalInput")
    ones2b_d = nc.dram_tensor("ones2b", [P, 2], bf16, kind="ExternalInput")
    onesb_d = nc.dram_tensor("onesb", [1, P], f32r, kind="ExternalInput")
    onesbb_d = nc.dram_tensor("onesbb", [1, P], bf16, kind="ExternalInput")
    zeros2_d = nc.dram_tensor("zeros2", [P, 2], f32r, kind="ExternalInput")
    invcnt_d = nc.dram_tensor("invcnt", [P, ROWS], f32, kind="ExternalInput")
    ninvcnt_d = nc.dram_tensor("ninvcnt", [P, ROWS], f32, kind="ExternalInput")
    if with_bias:
        bvec = nc.dram_tensor("bvec", [C, 1], f32, kind="ExternalInput")
    y = nc.dram_tensor("y", [C, T], f32, kind="ExternalOutput")

    with tile.TileContext(nc) as tc, ExitStack() as ctx:
        const = ctx.enter_context(tc.tile_pool(name="const", bufs=1))
        persist = ctx.enter_context(tc.tile_pool(name="persist", bufs=1))
        xpool = ctx.enter_context(tc.tile_pool(name="xpool", bufs=LAG + 1))
        ypool = ctx.enter_context(tc.tile_pool(name="ypool", bufs=2))
        sqpool = ctx.enter_context(tc.tile_pool(name="sqpool", bufs=2))
        erow = ctx.enter_context(tc.tile_pool(name="erow", bufs=3))
        brpool = ctx.enter_context(tc.tile_pool(name="brow", bufs=4))
        ps_s = ctx.enter_context(tc.tile_pool(name="ps_s", bufs=2, space="PSUM"))
        ps_nm = ctx.enter_context(tc.tile_pool(name="ps_nm", bufs=1, space="PSUM"))
        ps_i = ctx.enter_context(tc.tile_pool(name="ps_i", bufs=1, space="PSUM"))
        zpool = ctx.enter_context(tc.tile_pool(name="zpool", bufs=3))

        # ---- constants ----
        tri = const.tile([P, P], f32r)
        nc.sync.dma_start(out=tri, in_=tri_d[:, :])
        ones2r = const.tile([P, 2], f32r)
        nc.sync.dma_start(out=ones2r, in_=ones2r_d[:, :])
        ones2b = const.tile([P, 2], bf16)
        nc.sync.dma_start(out=ones2b, in_=ones2b_d[:, :])
        onesb = const.tile([1, P], f32r)
        nc.sync.dma_start(out=onesb, in_=onesb_d[:, :])
        onesbb = const.tile([1, P], bf16)
        nc.sync.dma_start(out=onesbb, in_=onesbb_d[:, :])
        invcnt = const.tile([P, ROWS], f32)
        nc.sync.dma_start(out=invcnt, in_=invcnt_d[:, :])
        ninvcnt = const.tile([P, ROWS], f32)
        nc.sync.dma_start(out=ninvcnt, in_=ninvcnt_d[:, :])
        w_sb = const.tile([P, NH], f32)
        for h in range(NH):
            nc.sync.dma_start(out=w_sb[:, h : h + 1], in_=wvec[h * P : (h + 1) * P, 0:1])
        if with_bias:
            b_sb = const.tile([P, NH], f32)
            for h in range(NH):
                nc.sync.dma_start(
                    out=b_sb[:, h : h + 1], in_=bvec[h * P : (h + 1) * P, 0:1]
                )
        eps_sb = const.tile([P, 1], f32)
        nc.vector.memset(eps_sb, EPS)

        # ---- persistent stat-layout surfaces ----
        s_re = persist.tile([P, ROWS], f32)     # channel sums -> prefix sums
        sq_re = persist.tile([P, ROWS], f32)
        nmist = persist.tile([P, 2, ROWS], bf16)  # plane 0: -mean, plane 1: istd
        ex2_t = persist.tile([P, ROWS], f32)    # E[x^2] -> var
        msq_t = persist.tile([P, ROWS], f32)    # mean^2 -> sqrt(var+eps)
        st_sb = persist.tile([P, 2], f32r)      # chunk totals (s, sq)
        nc.sync.dma_start(out=st_sb, in_=zeros2_d[:, :])

        tiles = {}

        def load_x(tix):
            """One 3D DMA for both halves of io-tile tix (issued from SP)."""
            t0 = tix * CHUNK
            x_t = xpool.tile([P, NH, CHUNK], f32r, tag="x", name="x_t")
            nc.sync.dma_start(
                out=x_t,
                in_=x.rearrange("(h p) t -> p h t", h=NH)[
                    :, :, t0 : t0 + CHUNK
                ].bitcast(f32r),
            )
            tiles[tix] = x_t

        def phase_a(tix):
            """Squares + channel sums/sumsq of io-tile tix into stat rows."""
            x_t = tiles[tix]
            xx0 = sqpool.tile([P, CHUNK], bf16, tag="xx0", name="xx0")
            nc.scalar.activation(xx0, x_t[:, 0, :].bitcast(f32), ACTF.Square)
            xx1 = sqpool.tile([P, CHUNK], bf16, tag="xx1", name="xx1")
            nc.gpsimd.tensor_tensor(
                xx1, x_t[:, 1, :].bitcast(f32), x_t[:, 1, :].bitcast(f32), ALU.mult
            )

            for a2 in range(2):  # 1000-col groups
                sps = ps_s.tile([2, 2, 512], f32, tag="stat", name="sps")
                qps = ps_s.tile([2, 2, 512], f32, tag="stat", name="qps")
                for j in range(2):
                    cs = slice((2 * a2 + j) * PB, (2 * a2 + j + 1) * PB)
                    nc.tensor.matmul(
                        sps[0:2, j, 0:PB], ones2r, x_t[:, 0, cs],
                        start=True, stop=False,
                    )
                    nc.tensor.matmul(
                        sps[0:2, j, 0:PB], ones2r, x_t[:, 1, cs],
                        start=False, stop=True,
                    )
                for j in range(2):
                    cs = slice((2 * a2 + j) * PB, (2 * a2 + j + 1) * PB)
                    nc.tensor.matmul(
                        qps[0:2, j, 0:PB], ones2b, xx0[:, cs], start=True, stop=False
                    )
                    nc.tensor.matmul(
                        qps[0:2, j, 0:PB], ones2b, xx1[:, cs], start=False, stop=True
                    )
                srow = erow.tile([1, 1024], f32, tag="erow", name="srow")
                nc.scalar.copy(
                    srow[0:1, 0:1000].rearrange("p (j n) -> p j n", j=2),
                    sps[0:1, :, 0:PB],
                )
                qrow = erow.tile([1, 1024], f32, tag="erow", name="qrow")
                nc.scalar.copy(
                    qrow[0:1, 0:1000].rearrange("p (j n) -> p j n", j=2),
                    qps[0:1, :, 0:PB],
                )
                # rows 16*tix+8*a2 .. +8 of the stat layout (t = 125*p + i);
                # reshape DMAs are cheap to issue from the GPSIMD queue.
                rp = 16 * tix + 8 * a2
                nc.gpsimd.dma_start(out=s_re[rp : rp + 8, :], in_=srow[0:1, 0:1000])
                nc.gpsimd.dma_start(out=sq_re[rp : rp + 8, :], in_=qrow[0:1, 0:1000])

        def stats(sc):
            """Prefix sums + nm/istd for stat-layout rows 32*sc .. 32*sc+32."""
            sl = slice(32 * sc, 32 * sc + 32)
            nc.vector.tensor_tensor_scan(
                out=s_re[sl, :], data0=s_re[sl, :], data1=s_re[sl, :],
                initial=0.0, op0=ALU.add, op1=ALU.bypass,
            )
            nc.vector.tensor_tensor_scan(
                out=sq_re[sl, :], data0=sq_re[sl, :], data1=sq_re[sl, :],
                initial=0.0, op0=ALU.add, op1=ALU.bypass,
            )
            nc.vector.tensor_copy(st_sb[sl, 0:1], s_re[sl, ROWS - 1 : ROWS])
            nc.vector.tensor_copy(st_sb[sl, 1:2], sq_re[sl, ROWS - 1 : ROWS])
            offps = ps_s.tile([P, 2], f32, tag="stat", name="offps")
            nc.tensor.matmul(offps, tri, st_sb, start=True, stop=True)

            # nm = -(s + off) / cnt  (negation folded into the constant).
            # bf16 output: |mean| error ~0.4% rel, far inside the 2e-2 gate.
            with nc.allow_low_precision(reason="bf16 nm/istd broadcast rows"):
                nc.vector.scalar_tensor_tensor(
                    out=nmist[sl, 0, :], in0=s_re[sl, :], scalar=offps[sl, 0:1],
                    in1=ninvcnt[sl, :], op0=ALU.add, op1=ALU.mult,
                )
            nc.vector.scalar_tensor_tensor(
                out=ex2_t[sl, :], in0=sq_re[sl, :], scalar=offps[sl, 1:2],
                in1=invcnt[sl, :], op0=ALU.add, op1=ALU.mult,
            )
            nc.vector.tensor_tensor(
                msq_t[sl, :], nmist[sl, 0, :], nmist[sl, 0, :], ALU.mult
            )
            nc.vector.tensor_tensor(ex2_t[sl, :], ex2_t[sl, :], msq_t[sl, :], ALU.subtract)
            # istd = 1 / sqrt(var + eps)  (Sqrt keeps the ACT table set stable)
            nc.scalar.activation(
                msq_t[sl, :], ex2_t[sl, :], ACTF.Sqrt, bias=eps_sb[sl, :], scale=1.0
            )
            with nc.allow_low_precision(reason="bf16 nm/istd broadcast rows"):
                nc.vector.reciprocal(out=nmist[sl, 1, :], in_=msq_t[sl, :])

        def gather(tix):
            """Two DMAs: the tile's 16 nm/istd stat rows -> brow planes
            [1, 2, 16, 125] (plane-major so broadcast rhs slices stay
            contiguous).  Issued from ACT, right behind the chunk's Sqrt."""
            rsl = slice(16 * tix, 16 * tix + 16)
            brow = brpool.tile([1, 2, 16, ROWS], bf16, tag="brow", name="brow")
            nc.scalar.dma_start(out=brow[:, 0, :, :], in_=nmist[rsl, 0, :])
            nc.scalar.dma_start(out=brow[:, 1, :, :], in_=nmist[rsl, 1, :])
            return brow

        def bc_mms(tix, brow, half):
            """PE rank-1 broadcasts for 1000-col half-tile `half`."""
            nm_ps = ps_nm.tile([P, 2, 512], f32, tag="nm", name="nm_ps")
            ibc = ps_i.tile([P, 2, 512], f32, tag="ibc_ps", name="ibc")
            for j in range(2):
                r0 = 8 * half + 4 * j
                nc.tensor.matmul(
                    nm_ps[:, j, 0:PB], onesbb,
                    brow[0:1, 0, r0 : r0 + 4, :],
                    start=True, stop=True,
                )
                nc.tensor.matmul(
                    ibc[:, j, 0:PB], onesbb,
                    brow[0:1, 1, r0 : r0 + 4, :],
                    start=True, stop=True,
                )
            return nm_ps, ibc

        def applies(tix, half, nm_ps, ibc, y_st):
            """DVE apply pair per channel half for this half-tile."""
            x_t = tiles[tix]
            for h in range(NH):
                x_ap = x_t[:, h, half * 1000 : (half + 1) * 1000].bitcast(
                    f32
                ).rearrange("p (j n) -> p j n", j=2)
                # z = x - mean  (one DVE op; nm_ps is the -mean broadcast)
                z_sb = zpool.tile([P, 2, PB], f32, tag="z", name="z_sb")
                nc.vector.scalar_tensor_tensor(
                    out=z_sb, in0=nm_ps[:, :, 0:PB], scalar=1.0,
                    in1=x_ap, op0=ALU.mult, op1=ALU.add,
                )
                # y = (z * w) * istd, istd read straight from PSUM
                y_ap = y_st[h][
                    :, half * 1000 : (half + 1) * 1000
                ].rearrange("p (j n) -> p j n", j=2)
                nc.vector.scalar_tensor_tensor(
                    out=y_ap, in0=z_sb, scalar=w_sb[:, h : h + 1],
                    in1=ibc[:, :, 0:PB], op0=ALU.mult, op1=ALU.mult,
                )
                if with_bias:
                    nc.vector.tensor_scalar_add(
                        out=y_ap, in0=y_ap, scalar1=b_sb[:, h : h + 1]
                    )

        # ---- software-pipelined emission ----
        # x-loads lead phase C by 4 tiles, A-compute by 3, and the stats
        # chain + gather for chunk sc run TWO rounds before C consumes them,
        # so phase C's broadcasts never wait on the serial chain.  Per-engine
        # FIFO orders per round k:
        #   SP:   y-stores(k-1), x-load(k+4)
        #   PE:   nmA(k), ibcA(k), A-mms(k+3), tri(sc), nmB(k), ibcB(k)
        #   ACT:  xx0(k+3), evacs(k+3), sqrt(sc), gathers(k+2, k+3)
        #   DVE:  applies halfA(k), chain(sc), applies halfB(k)
        #   Pool: xx1(k+3), stat DMAs(k+3)
        brows = {}
        ystore = {}

        def emit_stores(k):
            t0 = k * CHUNK
            y_st = ystore.pop(k)
            for h in range(NH):
                nc.sync.dma_start(
                    out=y[h * P : (h + 1) * P, t0 : t0 + CHUNK], in_=y_st[h]
                )

        for t in range(4):
            load_x(t)
        for t in range(3):
            phase_a(t)
        stats(0)
        brows[0] = gather(0)
        brows[1] = gather(1)

        for k in range(NCHUNK):
            if k >= 1:
                emit_stores(k - 1)
            if k + 4 < NCHUNK:
                load_x(k + 4)
            y_st = {
                h: ypool.tile([P, CHUNK], f32, tag=f"y{h}", name=f"y{h}")
                for h in range(NH)
            }
            ystore[k] = y_st
            brow = brows.pop(k)
            nmA, ibcA = bc_mms(k, brow, 0)
            if k + 3 < NCHUNK:
                phase_a(k + 3)
            applies(k, 0, nmA, ibcA, y_st)
            if k % 2 == 0 and k + 2 < NCHUNK:
                stats((k + 2) // 2)
            nmB, ibcB = bc_mms(k, brow, 1)
            applies(k, 1, nmB, ibcB, y_st)
            if k % 2 == 0 and k + 2 < NCHUNK:
                brows[k + 2] = gather(k + 2)
                brows[k + 3] = gather(k + 3)
            tiles.pop(k)
        emit_stores(NCHUNK - 1)
    nc.compile()
    return nc


def _consts():
    tri = np.triu(np.ones((P, P), dtype=np.float32), k=1)  # tri[k,m]=1 iff k<m
    ones2 = np.ones((P, 2), dtype=np.float32)
    onesb = np.ones((1, P), dtype=np.float32)
    t_idx = (125 * np.arange(P, dtype=np.float64)[:, None]
             + np.arange(ROWS, dtype=np.float64)[None, :])
    invcnt = (1.0 / (C * (t_idx + 1.0))).astype(np.float32)
    return {"tri": tri, "ones2r": ones2,
            "ones2b": ones2.astype(ml_dtypes.bfloat16), "onesb": onesb,
            "onesbb": onesb.astype(ml_dtypes.bfloat16),
            "zeros2": np.zeros((P, 2), dtype=np.float32),
            "invcnt": invcnt, "ninvcnt": -invcnt}


def _get_nc(with_bias: bool):
    key = ("nc", with_bias)
    if key not in _cached:
        _cached[key] = _build_nc(with_bias)
    return _cached[key]


def _run(x, weight, bias, trace=False):
    from concourse.bass_utils import run_bass_kernel_spmd

    x = np.ascontiguousarray(np.asarray(x, dtype=np.float32))
    weight = np.asarray(weight, dtype=np.float32).reshape(C, 1)
    bias = np.asarray(bias, dtype=np.float32).reshape(C, 1)
    with_bias = bool(np.any(bias))
    nc = _get_nc(with_bias)

    consts = _consts()
    in_maps = []
    for b in range(B):
        m = {"x": np.ascontiguousarray(x[b]), "wvec": weight}
        if with_bias:
            m["bvec"] = bias
        m.update(consts)
        in_maps.append(m)

    res = run_bass_kernel_spmd(nc, in_maps, core_ids=list(range(B)), trace=trace)
    y = np.stack([r["y"] for r in res.results], axis=0)
    return y, res


def kernel(x, weight, bias):
    y, _ = _run(x, weight, bias, trace=False)
    return y


# revision 10
# speedup vs baseline: 1.2953x; 1.0752x over previous
"""Cumulative LayerNorm Trainium2 Bass kernel.

x: [B=8, C=256, T=16000] f32.  Per timestep t: normalize x[:, :, t] by the
mean/std of all elements x[:, :, t'<=t] (cumulative over channels+time), then
scale by weight[c] and add bias[c].

Sharding: pure data parallel over B across 8 NeuronCores (1 sample/core).

Per-core algorithm (C=256 = 2 halves of 128 partitions, T on the free dim):
  Phase A (per 2000-col io-tile):
    - One 3D DMA loads both channel halves into SBUF [128, 2, 2000]
      (labeled f32r so the PE consumes it directly; fp32r truncates operands
      to ~13 mantissa bits inside the PE only).
    - xx = x^2 in bf16 (ACT for half 0, GPSIMD for half 1).
    - PE: s[t] = sum_c x (fp32r ones weights) and sq[t] = sum_c x^2 (bf16)
      as [2, 2, 512] PSUM row-blocks per 1000-col group; ACT evacuates row 0
      to a [1, 1000] SBUF row; a reshape DMA (issued from GPSIMD, cheap
      there) scatters it into the [128, 125] "stat layout" where t = 125p+i.
  Stats (per 4000-col chunk = 32 stat rows; engine ops need 32-aligned
  partition bases):
    - DVE tensor_tensor_scan along i (per-partition prefix sums).
    - Row totals go to st[128, 2]; a strict-upper-triangular fp32r matmul
      gives exclusive cross-partition offsets (future rows zeroed, so one
      full-K matmul per chunk is exact).
    - nm = -(scan + off) * invcnt directly (negation folded into the
      constant); var = E[x^2] - nm^2; istd = 1/sqrt(var + eps) (ACT Sqrt +
      DVE reciprocal).  nm and istd land in one [128, 2, 125] surface.
  Phase C (per io-tile):
    - ONE gather DMA (issued from ACT right after the chunk chain) pulls the
      tile's 16 stat rows into a row buffer brow[1, 16, 2, 125].
    - Per 1000-col half-tile: PE rank-1 broadcasts nm_bc and ibc into PSUM.
    - DVE scalar_tensor_tensor pair per channel half: z = nm_bc + x;
      y = (z * w[p]) * ibc -- the second STT reads ibc straight from PSUM
      (no ACT copy).  y lands in a per-(tile, half) [128, 2000] staging tile
      stored with one DMA.

Emission is software-pipelined at io-tile granularity (phase C lags the
x-loads by 4 tiles) and each engine's FIFO is ordered so streaming work never
queues behind the long-latency stats chain.
"""
import ml_dtypes
import numpy as np

B, C, T = 8, 256, 16000
P = 128
NH = 2                     # channel halves
CHUNK = 2000               # t per io-tile
NCHUNK = T // CHUNK        # 8
ROWS = T // P              # 125  (stat layout free dim; t = 125*p + i)
PB = 500                   # psum block columns (4 per io-tile)
LAG = 4                    # x-load leads phase C by this many tiles
EPS = 1e-06

_cached = {}


def _build_nc(with_bias: bool):
    from contextlib import ExitStack

    import concourse.tile as tile
    from concourse import bacc, mybir

    f32 = mybir.dt.float32
    f32r = mybir.dt.float32r
    bf16 = mybir.dt.bfloat16
    ALU = mybir.AluOpType
    ACTF = mybir.ActivationFunctionType

    nc = bacc.Bacc()

    x = nc.dram_tensor("x", [C, T], f32, kind="ExternalInput")
    wvec = nc.dram_tensor("wvec", [C, 1], f32, kind="ExternalInput")
    tri_d = nc.dram_tensor("tri", [P, P], f32r, kind="ExternalInput")
    ones2r_d = nc.dram_tensor("ones2r", [P, 2], f32r, kind="ExternalInput")
    ones2b_d = nc.dram_tensor("ones2b", [P, 2], bf16, kind="ExternalInput")
    onesb_d = nc.dram_tensor("onesb", [1, P], f32r, kind="ExternalInput")
    onesbb_d = nc.dram_tensor("onesbb", [1, P], bf16, kind="ExternalInput")
    zeros2_d = nc.dram_tensor("zeros2", [P, 2], f32r, kind="ExternalInput")
    invcnt_d = nc.dram_tensor("invcnt", [P, ROWS], f32, kind="ExternalInput")
    ninvcnt_d = nc.dram_tensor("ninvcnt", [P, ROWS], f32, kind="ExternalInput")
    if with_bias:
        bvec = nc.dram_tensor("bvec", [C, 1], f32, kind="ExternalInput")
    y = nc.dram_tensor("y", [C, T], f32, kind="ExternalOutput")

    with tile.TileContext(nc) as tc, ExitStack() as ctx:
        const = ctx.enter_context(tc.tile_pool(name="const", bufs=1))
        persist = ctx.enter_context(tc.tile_pool(name="persist", bufs=1))
        xpool = ctx.enter_context(tc.tile_pool(name="xpool", bufs=LAG + 1))
        ypool = ctx.enter_context(tc.tile_pool(name="ypool", bufs=2))
        sqpool = ctx.enter_context(tc.tile_pool(name="sqpool", bufs=2))
        erow = ctx.enter_context(tc.tile_pool(name="erow", bufs=3))
        brpool = ctx.enter_context(tc.tile_pool(name="brow", bufs=4))
        ps_s = ctx.enter_context(tc.tile_pool(name="ps_s", bufs=2, space="PSUM"))
        ps_nm = ctx.enter_context(tc.tile_pool(name="ps_nm", bufs=2, space="PSUM"))
        ps_i = ctx.enter_context(tc.tile_pool(name="ps_i", bufs=2, space="PSUM"))
        zpool = ctx.enter_context(tc.tile_pool(name="zpool", bufs=3))

        # ---- constants (issued from DVE: it is idle all warmup, and this
        # keeps the Sync queue free so x-loads trigger immediately) ----
        tri = const.tile([P, P], f32r)
        nc.gpsimd.dma_start(out=tri, in_=tri_d[:, :])
        ones2r = const.tile([P, 2], f32r)
        nc.gpsimd.dma_start(out=ones2r, in_=ones2r_d[:, :])
        ones2b = const.tile([P, 2], bf16)
        nc.gpsimd.dma_start(out=ones2b, in_=ones2b_d[:, :])
        onesb = const.tile([1, P], f32r)
        nc.gpsimd.dma_start(out=onesb, in_=onesb_d[:, :])
        onesbb = const.tile([1, P], bf16)
        nc.gpsimd.dma_start(out=onesbb, in_=onesbb_d[:, :])
        invcnt = const.tile([P, ROWS], f32)
        nc.gpsimd.dma_start(out=invcnt, in_=invcnt_d[:, :])
        ninvcnt = const.tile([P, ROWS], f32)
        nc.gpsimd.dma_start(out=ninvcnt, in_=ninvcnt_d[:, :])
        w_sb = const.tile([P, NH], f32)
        for h in range(NH):
            nc.gpsimd.dma_start(out=w_sb[:, h : h + 1], in_=wvec[h * P : (h + 1) * P, 0:1])
        if with_bias:
            b_sb = const.tile([P, NH], f32)
            for h in range(NH):
                nc.gpsimd.dma_start(
                    out=b_sb[:, h : h + 1], in_=bvec[h * P : (h + 1) * P, 0:1]
                )
        eps_sb = const.tile([P, 1], f32)
        nc.vector.memset(eps_sb, EPS)

        # ---- persistent stat-layout surfaces ----
        s_re = persist.tile([P, ROWS], f32)     # channel sums -> prefix sums
        sq_re = persist.tile([P, ROWS], f32)
        nmist = persist.tile([P, 2, ROWS], bf16)  # plane 0: -mean, plane 1: istd
        ex2_t = persist.tile([P, ROWS], f32)    # E[x^2] -> var
        msq_t = persist.tile([P, ROWS], f32)    # mean^2 -> sqrt(var+eps)
        st_sb = persist.tile([P, 2], f32r)      # chunk totals (s, sq)
        nc.gpsimd.dma_start(out=st_sb, in_=zeros2_d[:, :])

        tiles = {}

        def load_x(tix):
            """One 3D DMA for both halves of io-tile tix (issued from SP)."""
            t0 = tix * CHUNK
            x_t = xpool.tile([P, NH, CHUNK], f32r, tag="x", name="x_t")
            nc.sync.dma_start(
                out=x_t,
                in_=x.rearrange("(h p) t -> p h t", h=NH)[
                    :, :, t0 : t0 + CHUNK
                ].bitcast(f32r),
            )
            tiles[tix] = x_t

        sq_tiles = {}

        def squares(tix, act_both=False):
            """x^2 in bf16: ACT for half 0, GPSIMD for half 1 (or both on ACT
            during warmup, when GPSIMD serializing would delay the chain)."""
            x_t = tiles[tix]
            xx0 = sqpool.tile([P, CHUNK], bf16, tag="xx0", name="xx0")
            nc.scalar.activation(xx0, x_t[:, 0, :].bitcast(f32), ACTF.Square)
            xx1 = sqpool.tile([P, CHUNK], bf16, tag="xx1", name="xx1")
            if act_both:
                nc.scalar.activation(xx1, x_t[:, 1, :].bitcast(f32), ACTF.Square)
            else:
                nc.gpsimd.tensor_tensor(
                    xx1, x_t[:, 1, :].bitcast(f32), x_t[:, 1, :].bitcast(f32),
                    ALU.mult,
                )
            sq_tiles[tix] = (xx0, xx1)

        def a_sums_s(tix):
            """Channel sums of x -> stat rows (PE mms, ACT evac, Pool DMA)."""
            x_t = tiles[tix]
            for a2 in range(2):  # 1000-col groups
                sps = ps_s.tile([2, 2, 512], f32, tag="stat", name="sps")
                for j in range(2):
                    cs = slice((2 * a2 + j) * PB, (2 * a2 + j + 1) * PB)
                    nc.tensor.matmul(
                        sps[0:2, j, 0:PB], ones2r, x_t[:, 0, cs],
                        start=True, stop=False,
                    )
                    nc.tensor.matmul(
                        sps[0:2, j, 0:PB], ones2r, x_t[:, 1, cs],
                        start=False, stop=True,
                    )
                srow = erow.tile([1, 1024], f32, tag="erow", name="srow")
                nc.scalar.copy(
                    srow[0:1, 0:1000].rearrange("p (j n) -> p j n", j=2),
                    sps[0:1, :, 0:PB],
                )
                rp = 16 * tix + 8 * a2
                nc.gpsimd.dma_start(out=s_re[rp : rp + 8, :], in_=srow[0:1, 0:1000])

        def a_sums_q(tix):
            """Channel sums of x^2 -> stat rows."""
            xx0, xx1 = sq_tiles.pop(tix)
            for a2 in range(2):
                qps = ps_s.tile([2, 2, 512], f32, tag="stat", name="qps")
                for j in range(2):
                    cs = slice((2 * a2 + j) * PB, (2 * a2 + j + 1) * PB)
                    nc.tensor.matmul(
                        qps[0:2, j, 0:PB], ones2b, xx0[:, cs], start=True, stop=False
                    )
                    nc.tensor.matmul(
                        qps[0:2, j, 0:PB], ones2b, xx1[:, cs], start=False, stop=True
                    )
                qrow = erow.tile([1, 1024], f32, tag="erow", name="qrow")
                nc.scalar.copy(
                    qrow[0:1, 0:1000].rearrange("p (j n) -> p j n", j=2),
                    qps[0:1, :, 0:PB],
                )
                rp = 16 * tix + 8 * a2
                nc.gpsimd.dma_start(out=sq_re[rp : rp + 8, :], in_=qrow[0:1, 0:1000])

        def stats(sc):
            """Prefix sums + nm/istd for stat-layout rows 32*sc .. 32*sc+32."""
            sl = slice(32 * sc, 32 * sc + 32)
            nc.vector.tensor_tensor_scan(
                out=s_re[sl, :], data0=s_re[sl, :], data1=s_re[sl, :],
                initial=0.0, op0=ALU.add, op1=ALU.bypass,
            )
            nc.vector.tensor_tensor_scan(
                out=sq_re[sl, :], data0=sq_re[sl, :], data1=sq_re[sl, :],
                initial=0.0, op0=ALU.add, op1=ALU.bypass,
            )
            nc.vector.tensor_copy(st_sb[sl, 0:1], s_re[sl, ROWS - 1 : ROWS])
            nc.vector.tensor_copy(st_sb[sl, 1:2], sq_re[sl, ROWS - 1 : ROWS])
            offps = ps_s.tile([P, 2], f32, tag="stat", name="offps")
            nc.tensor.matmul(offps, tri, st_sb, start=True, stop=True)

            # nm = -(s + off) / cnt  (negation folded into the constant).
            # bf16 output: |mean| error ~0.4% rel, far inside the 2e-2 gate.
            with nc.allow_low_precision(reason="bf16 nm/istd broadcast rows"):
                nc.vector.scalar_tensor_tensor(
                    out=nmist[sl, 0, :], in0=s_re[sl, :], scalar=offps[sl, 0:1],
                    in1=ninvcnt[sl, :], op0=ALU.add, op1=ALU.mult,
                )
            nc.vector.scalar_tensor_tensor(
                out=ex2_t[sl, :], in0=sq_re[sl, :], scalar=offps[sl, 1:2],
                in1=invcnt[sl, :], op0=ALU.add, op1=ALU.mult,
            )
            nc.vector.tensor_tensor(
                msq_t[sl, :], nmist[sl, 0, :], nmist[sl, 0, :], ALU.mult
            )
            nc.vector.tensor_tensor(ex2_t[sl, :], ex2_t[sl, :], msq_t[sl, :], ALU.subtract)
            # istd = 1 / sqrt(var + eps)  (Sqrt keeps the ACT table set stable)
            nc.scalar.activation(
                msq_t[sl, :], ex2_t[sl, :], ACTF.Sqrt, bias=eps_sb[sl, :], scale=1.0
            )
            with nc.allow_low_precision(reason="bf16 nm/istd broadcast rows"):
                nc.vector.reciprocal(out=nmist[sl, 1, :], in_=msq_t[sl, :])

        def gather(tix):
            """Two DMAs: the tile's 16 nm/istd stat rows -> brow planes
            [1, 2, 16, 125] (plane-major so broadcast rhs slices stay
            contiguous).  Issued from ACT, right behind the chunk's Sqrt."""
            rsl = slice(16 * tix, 16 * tix + 16)
            brow = brpool.tile([1, 2, 16, ROWS], bf16, tag="brow", name="brow")
            nc.scalar.dma_start(out=brow[:, 0, :, :], in_=nmist[rsl, 0, :])
            nc.scalar.dma_start(out=brow[:, 1, :, :], in_=nmist[rsl, 1, :])
            return brow

        def bc_mms(tix, brow, blk):
            """PE rank-1 broadcasts for 500-col block `blk` (0..3)."""
            nm_ps = ps_nm.tile([P, 512], f32, tag="nm", name="nm_ps")
            ibc = ps_i.tile([P, 512], f32, tag="ibc_ps", name="ibc")
            r0 = 4 * blk
            nc.tensor.matmul(
                nm_ps[:, 0:PB], onesbb, brow[0:1, 0, r0 : r0 + 4, :],
                start=True, stop=True,
            )
            nc.tensor.matmul(
                ibc[:, 0:PB], onesbb, brow[0:1, 1, r0 : r0 + 4, :],
                start=True, stop=True,
            )
            return nm_ps, ibc

        def applies(tix, blk, nm_ps, ibc, y_st):
            """DVE apply pair per channel half for 500-col block `blk`."""
            x_t = tiles[tix]
            cs = slice(blk * PB, (blk + 1) * PB)
            for h in range(NH):
                # z = x - mean  (one DVE op; nm_ps is the -mean broadcast)
                z_sb = zpool.tile([P, PB], f32, tag="z", name="z_sb")
                nc.vector.scalar_tensor_tensor(
                    out=z_sb, in0=nm_ps[:, 0:PB], scalar=1.0,
                    in1=x_t[:, h, cs].bitcast(f32), op0=ALU.mult, op1=ALU.add,
                )
                # y = (z * w) * istd, istd read straight from PSUM
                y_ap = y_st[h][:, cs]
                nc.vector.scalar_tensor_tensor(
                    out=y_ap, in0=z_sb, scalar=w_sb[:, h : h + 1],
                    in1=ibc[:, 0:PB], op0=ALU.mult, op1=ALU.mult,
                )
                if with_bias:
                    nc.vector.tensor_scalar_add(
                        out=y_ap, in0=y_ap, scalar1=b_sb[:, h : h + 1]
                    )

        # ---- software-pipelined emission ----
        # x-loads lead phase C by 4 tiles, A-compute by 3, and the stats
        # chain + gather for chunk sc run TWO rounds before C consumes them,
        # so phase C's broadcasts never wait on the serial chain.  Per-engine
        # FIFO orders per round k:
        #   SP:   y-stores(k-1), x-load(k+4)
        #   PE:   nmA(k), ibcA(k), A-mms(k+3), tri(sc), nmB(k), ibcB(k)
        #   ACT:  xx0(k+3), evacs(k+3), sqrt(sc), gathers(k+2, k+3)
        #   DVE:  applies halfA(k), chain(sc), applies halfB(k)
        #   Pool: xx1(k+3), stat DMAs(k+3)
        brows = {}
        ystore = {}

        def emit_stores(k):
            t0 = k * CHUNK
            y_st = ystore.pop(k)
            for h in range(NH):
                nc.sync.dma_start(
                    out=y[h * P : (h + 1) * P, t0 : t0 + CHUNK], in_=y_st[h]
                )

        # Prologue: loads first so x(0) streams immediately; tiles 0-1 square
        # on ACT only (GPSIMD's slow multiply would delay the first chain).
        for t in range(4):
            load_x(t)
        squares(0, act_both=True)
        a_sums_s(0)
        squares(1, act_both=True)
        a_sums_s(1)
        a_sums_q(0)
        a_sums_q(1)
        stats(0)
        squares(2)
        a_sums_s(2)
        a_sums_q(2)
        brows[0] = gather(0)
        brows[1] = gather(1)

        for k in range(NCHUNK):
            if k >= 1:
                emit_stores(k - 1)
            if k + 4 < NCHUNK:
                load_x(k + 4)
            y_st = {
                h: ypool.tile([P, CHUNK], f32, tag=f"y{h}", name=f"y{h}")
                for h in range(NH)
            }
            ystore[k] = y_st
            brow = brows.pop(k)
            if k + 3 < NCHUNK:
                squares(k + 3)
            bc0 = bc_mms(k, brow, 0)
            bc1 = bc_mms(k, brow, 1)
            applies(k, 0, *bc0, y_st)
            if k + 3 < NCHUNK:
                a_sums_s(k + 3)
            applies(k, 1, *bc1, y_st)
            bc2 = bc_mms(k, brow, 2)
            if k + 3 < NCHUNK:
                a_sums_q(k + 3)
            if k % 2 == 0 and k + 2 < NCHUNK:
                stats((k + 2) // 2)
            applies(k, 2, *bc2, y_st)
            bc3 = bc_mms(k, brow, 3)
            applies(k, 3, *bc3, y_st)
            if k % 2 == 0 and k + 2 < NCHUNK:
                brows[k + 2] = gather(k + 2)
                brows[k + 3] = gather(k + 3)
            tiles.pop(k)
        emit_stores(NCHUNK - 1)
    nc.compile()
    return nc


def _consts():
    tri = np.triu(np.ones((P, P), dtype=np.float32), k=1)  # tri[k,m]=1 iff k<m
    ones2 = np.ones((P, 2), dtype=np.float32)
    onesb = np.ones((1, P), dtype=np.float32)
    t_idx = (125 * np.arange(P, dtype=np.float64)[:, None]
             + np.arange(ROWS, dtype=np.float64)[None, :])
    invcnt = (1.0 / (C * (t_idx + 1.0))).astype(np.float32)
    return {"tri": tri, "ones2r": ones2,
            "ones2b": ones2.astype(ml_dtypes.bfloat16), "onesb": onesb,
            "onesbb": onesb.astype(ml_dtypes.bfloat16),
            "zeros2": np.zeros((P, 2), dtype=np.float32),
            "invcnt": invcnt, "ninvcnt": -invcnt}


def _get_nc(with_bias: bool):
    key = ("nc", with_bias)
    if key not in _cached:
        _cached[key] = _build_nc(with_bias)
    return _cached[key]


def _run(x, weight, bias, trace=False):
    from concourse.bass_utils import run_bass_kernel_spmd

    x = np.ascontiguousarray(np.asarray(x, dtype=np.float32))
    weight = np.asarray(weight, dtype=np.float32).reshape(C, 1)
    bias = np.asarray(bias, dtype=np.float32).reshape(C, 1)
    with_bias = bool(np.any(bias))
    nc = _get_nc(with_bias)

    consts = _consts()
    in_maps = []
    for b in range(B):
        m = {"x": np.ascontiguousarray(x[b]), "wvec": weight}
        if with_bias:
            m["bvec"] = bias
        m.update(consts)
        in_maps.append(m)

    res = run_bass_kernel_spmd(nc, in_maps, core_ids=list(range(B)), trace=trace)
    y = np.stack([r["y"] for r in res.results], axis=0)
    return y, res


def kernel(x, weight, bias):
    y, _ = _run(x, weight, bias, trace=False)
    return y
